# revision 4
# baseline (speedup 1.0000x reference)
"""Multi-head attention kernel for 8 Trainium2 NeuronCores.

Problem: B=4, L=2048, DIM=1024, H=16 heads, d_k=d_v=64.
  qh = q @ Wq_h ; kh = k @ Wk_h ; vh = v @ Wv_h          (per head)
  out_h = softmax(qh kh^T / sqrt(DIM)) vh
  y = concat_h(out_h) @ proj_w.T + proj_b

Sharding: data-parallel over (batch, query-half): core c handles batch
c//2, query rows [1024*(c%2), 1024*(c%2)+1024). Each core holds the full
K/V for its batch, so there are no collectives; K/V projections are
computed twice per batch (12% extra flops) in exchange for zero comms.

Device dataflow (per core, everything transposed so the contraction dim
sits on SBUF partitions):
  phase V: VH'[st] [128s, 16h, 65] <- vT chunks (stationary) @ wv  (+ ones col)
  phase Q: QHT[hp] [128(2 heads*dk), 1024q] <- wq chunks @ qT
  phase K: KHT[hp] [128, 2048s]            <- wk chunks @ kT
  phase B per head: S^T[key,q] = KHT_h^T-slice @ QHT_h-slice (K=64)
      expS = exp(S^T/32) (bf16) ; O'^T[65,q] = sum_st VH'_h[st]^T @ expS[st]
      row 64 of O'^T = softmax denominator; reciprocal -> DRAM bounce ->
      partition-broadcast -> O^T = O'^T[0:64] * recip  -> oall[h] [64,1024]
  phase C: yT[d,t] = sum_h pwT_h^T @ oall[h]  (K=64 chunks) + bias

Matmul dtypes: float32r (fp22, full-rate) for projections/output proj,
bf16 for scores/PV inputs. PSUM accumulation is always fp32.
"""

import numpy as np

P = 128
B, L, DIM, H, DK = 4, 2048, 1024, 16, 64
TQ = 1024      # q tokens per core
TS = 2048      # kv tokens per core
NDCH = DIM // P          # 8 contraction chunks
NHP = H // 2             # 8 head pairs
NST = TS // P            # 16 key tiles
N_CORES = 8

_NC = None
TRACE = False
LAST_RESULT = None


def _build():
    import concourse.bass as bass
    from concourse import bacc
    import concourse.mybir as mybir
    import concourse.tile as tile

    DT_R = mybir.dt.float32r
    DT_B = mybir.dt.bfloat16
    DT_F = mybir.dt.float32
    AF = mybir.ActivationFunctionType

    nc = bacc.Bacc(None, target_bir_lowering=False)
    qT = nc.dram_tensor("qT", [DIM, TQ], DT_R, kind="ExternalInput")
    kT = nc.dram_tensor("kT", [DIM, TS], DT_R, kind="ExternalInput")
    vT = nc.dram_tensor("vT", [DIM, TS], DT_R, kind="ExternalInput")
    wq = nc.dram_tensor("wq", [DIM, H * DK], DT_R, kind="ExternalInput")
    wk = nc.dram_tensor("wk", [DIM, H * DK], DT_R, kind="ExternalInput")
    wv = nc.dram_tensor("wv", [DIM, H * DK], DT_R, kind="ExternalInput")
    pw = nc.dram_tensor("pwT", [H * DK, DIM], DT_B, kind="ExternalInput")
    pb = nc.dram_tensor("pb", [P, NDCH], DT_F, kind="ExternalInput")
    yT = nc.dram_tensor("yT", [DIM, TQ], DT_F, kind="ExternalOutput")

    def bcast_ap(ap, count):
        return bass.AP(tensor=ap.tensor, offset=ap.offset,
                       ap=[[0, count]] + [list(x) for x in ap.ap[1:]])

    with tile.TileContext(nc) as tc, \
         tc.tile_pool(name="persist", bufs=1) as pp:
        # ---- persistent tiles (one pool, distinct tags, closes last) ----
        qht = [pp.tile([P, TQ], DT_B, name=f"qht{i}") for i in range(NHP)]
        kht = [pp.tile([P, TS], DT_B, name=f"kht{i}") for i in range(NHP)]
        vhp = [pp.tile([P, H, DK + 1], DT_B, name=f"vhp{i}") for i in range(NST)]
        oall = [pp.tile([DK, TQ], DT_B, name=f"oall{i}") for i in range(H)]
        pbt = pp.tile([P, NDCH], DT_F, name="pbt")
        nc.sync.dma_start(out=pbt[:, :], in_=pb[:, :])

        # ---- phase V: VH' = vT^T @ wv, tokens on partitions ----
        with tc.tile_pool(name="wv_pool", bufs=1) as wvp, \
             tc.tile_pool(name="vslab", bufs=2) as vsp, \
             tc.tile_pool(name="vpsum", bufs=2, space="PSUM") as vps:
            wvt = [wvp.tile([P, H * DK], DT_R, name=f"wvt{d}") for d in range(NDCH)]
            for d in range(NDCH):
                nc.sync.dma_start(out=wvt[d][:, :], in_=wv[d * P:(d + 1) * P, :])
            for st in range(NST):
                vsl = [vsp.tile([P, P], DT_R, name=f"vsl{d}", tag=f"vsl{d}")
                       for d in range(NDCH)]
                for d in range(NDCH):
                    nc.sync.dma_start(out=vsl[d][:, :],
                                      in_=vT[d * P:(d + 1) * P, st * P:(st + 1) * P])
                ps = [vps.tile([P, 512], DT_F, name=f"vps_{st}_{n}", tag=f"vps{n}")
                      for n in range(2)]
                for d in range(NDCH):
                    for n in range(2):
                        nc.tensor.matmul(ps[n][:, :], vsl[d][:, :],
                                         wvt[d][:, n * 512:(n + 1) * 512],
                                         start=(d == 0), stop=(d == NDCH - 1))
                for n in range(2):
                    nc.vector.tensor_copy(
                        vhp[st][:, n * 8:(n + 1) * 8, 0:DK],
                        ps[n][:, :].rearrange("p (h d) -> p h d", d=DK))
                nc.vector.memset(vhp[st][:, :, DK:DK + 1], 1.0)

        # ---- phase Q: QHT = wq^T @ qT ----
        with tc.tile_pool(name="qin_pool", bufs=1) as qip, \
             tc.tile_pool(name="wq_pool", bufs=1) as wqp, \
             tc.tile_pool(name="qpsum", bufs=2, space="PSUM") as qps:
            qin = [qip.tile([P, TQ], DT_R, name=f"qin{d}") for d in range(NDCH)]
            wqt = [wqp.tile([P, H * DK], DT_R, name=f"wqt{d}") for d in range(NDCH)]
            for d in range(NDCH):
                nc.sync.dma_start(out=qin[d][:, :], in_=qT[d * P:(d + 1) * P, :])
                nc.sync.dma_start(out=wqt[d][:, :], in_=wq[d * P:(d + 1) * P, :])
            for hp in range(NHP):
                ps = [qps.tile([P, 512], DT_F, name=f"qps_{hp}_{n}", tag=f"qps{n}")
                      for n in range(2)]
                for d in range(NDCH):
                    for n in range(2):
                        nc.tensor.matmul(ps[n][:, :], wqt[d][:, hp * P:(hp + 1) * P],
                                         qin[d][:, n * 512:(n + 1) * 512],
                                         start=(d == 0), stop=(d == NDCH - 1))
                for n in range(2):
                    nc.vector.tensor_copy(qht[hp][:, n * 512:(n + 1) * 512],
                                          ps[n][:, :])

        # ---- phase K: KHT = wk^T @ kT (two halves to bound SBUF) ----
        with tc.tile_pool(name="wk_pool", bufs=1) as wkp:
            wkt = [wkp.tile([P, H * DK], DT_R, name=f"wkt{d}") for d in range(NDCH)]
            for d in range(NDCH):
                nc.sync.dma_start(out=wkt[d][:, :], in_=wk[d * P:(d + 1) * P, :])
            for kh in range(2):
                with tc.tile_pool(name=f"kin_pool{kh}", bufs=1) as kip, \
                     tc.tile_pool(name=f"kpsum{kh}", bufs=2, space="PSUM") as kps:
                    kin = [kip.tile([P, 1024], DT_R, name=f"kin{kh}_{d}")
                           for d in range(NDCH)]
                    for d in range(NDCH):
                        nc.sync.dma_start(
                            out=kin[d][:, :],
                            in_=kT[d * P:(d + 1) * P, kh * 1024:(kh + 1) * 1024])
                    for hp in range(NHP):
                        ps = [kps.tile([P, 512], DT_F, name=f"kps_{kh}_{hp}_{n}",
                                       tag=f"kps{n}") for n in range(2)]
                        for d in range(NDCH):
                            for n in range(2):
                                nc.tensor.matmul(ps[n][:, :], wkt[d][:, hp * P:(hp + 1) * P],
                                                 kin[d][:, n * 512:(n + 1) * 512],
                                                 start=(d == 0), stop=(d == NDCH - 1))
                        for n in range(2):
                            nc.vector.tensor_copy(
                                kht[hp][:, kh * 1024 + n * 512:kh * 1024 + (n + 1) * 512],
                                ps[n][:, :])

        # ---- phase B: attention per head ----
        with tc.tile_pool(name="exp_pool", bufs=24) as expp, \
             tc.tile_pool(name="spsum", bufs=3, space="PSUM") as sps, \
             tc.tile_pool(name="opsum", bufs=2, space="PSUM") as ops, \
             tc.tile_pool(name="sums_pool", bufs=4) as smp, \
             tc.tile_pool(name="bc_pool", bufs=4) as bcp, \
             tc.tile_pool(name="bounce", bufs=4, space="DRAM") as bncp:

            exp_tiles = [None] * H

            def emit_scores(h):
                hp, hf = h // 2, h % 2
                lo, hi = hf * DK, (hf + 1) * DK
                tiles = []
                for kt in range(NST):
                    sp = sps.tile([P, TQ], DT_F, name=f"sps_{h}_{kt}", tag="sps")
                    for n in range(2):
                        nc.tensor.matmul(
                            sp[:, n * 512:(n + 1) * 512],
                            kht[hp][lo:hi, kt * P:(kt + 1) * P],
                            qht[hp][lo:hi, n * 512:(n + 1) * 512],
                            start=True, stop=True)
                    ex = expp.tile([P, TQ], DT_B, name=f"exp_{h}_{kt}", tag="exp")
                    nc.scalar.activation(ex[:, :], sp[:, :], AF.Exp, scale=1.0 / 32.0)
                    tiles.append(ex)
                exp_tiles[h] = tiles

            def emit_pv(h):
                tiles = exp_tiles[h]
                for qh in range(2):
                    op = ops.tile([P, 512], DT_F, name=f"ops_{h}_{qh}", tag="ops")
                    for kt in range(NST):
                        nc.tensor.matmul(op[0:DK + 1, :], vhp[kt][:, h, :],
                                         tiles[kt][:, qh * 512:(qh + 1) * 512],
                                         start=(kt == 0), stop=(kt == NST - 1))
                    sm = smp.tile([DK + 1, 512], DT_F, name=f"sm_{h}_{qh}", tag="sm")
                    nc.vector.reciprocal(sm[DK:DK + 1, :], op[DK:DK + 1, :])
                    bn = bncp.tile([1, 512], DT_F, name=f"bn_{h}_{qh}", tag="bn")
                    nc.sync.dma_start(out=bn[:, :], in_=sm[DK:DK + 1, :])
                    bc = bcp.tile([DK, 512], DT_F, name=f"bc_{h}_{qh}", tag="bc")
                    nc.sync.dma_start(out=bc[:, :], in_=bcast_ap(bn[0:1, :], DK))
                    nc.vector.tensor_mul(oall[h][:, qh * 512:(qh + 1) * 512],
                                         op[0:DK, :], bc[:, :])
                exp_tiles[h] = None

            emit_scores(0)
            for h in range(H):
                if h + 1 < H:
                    emit_scores(h + 1)
                emit_pv(h)

        # ---- phase C: yT = pwT^T @ oall + bias ----
        with tc.tile_pool(name="pw_pool", bufs=6) as pwp, \
             tc.tile_pool(name="ypsum", bufs=2, space="PSUM") as yps, \
             tc.tile_pool(name="yst_pool", bufs=4) as ystp:
            for dt_ in range(NDCH):
                ps = [yps.tile([P, 512], DT_F, name=f"yps_{dt_}_{n}", tag=f"yps{n}")
                      for n in range(2)]
                for h in range(H):
                    pwt = pwp.tile([DK, P], DT_B, name=f"pwt_{dt_}_{h}", tag="pw")
                    nc.sync.dma_start(out=pwt[:, :],
                                      in_=pw[h * DK:(h + 1) * DK,
                                             dt_ * P:(dt_ + 1) * P])
                    for n in range(2):
                        nc.tensor.matmul(ps[n][:, :], pwt[:, :],
                                         oall[h][:, n * 512:(n + 1) * 512],
                                         start=(h == 0), stop=(h == H - 1))
                for n in range(2):
                    yst = ystp.tile([P, 512], DT_F, name=f"yst_{dt_}_{n}", tag="yst")
                    nc.vector.tensor_scalar_add(yst[:, :], ps[n][:, :],
                                                pbt[:, dt_:dt_ + 1])
                    nc.sync.dma_start(
                        out=yT[dt_ * P:(dt_ + 1) * P, n * 512:(n + 1) * 512],
                        in_=yst[:, :])

    nc.compile()
    return nc


def kernel(q, k, v, w_q, w_k, w_v, proj_w, proj_b):
    global _NC, LAST_RESULT
    import ml_dtypes
    from concourse.bass_utils import run_bass_kernel_spmd

    if _NC is None:
        _NC = _build()

    q = np.asarray(q, dtype=np.float32)
    k = np.asarray(k, dtype=np.float32)
    v = np.asarray(v, dtype=np.float32)
    w_q = np.asarray(w_q, dtype=np.float32)
    w_k = np.asarray(w_k, dtype=np.float32)
    w_v = np.asarray(w_v, dtype=np.float32)
    proj_w = np.asarray(proj_w, dtype=np.float32)
    proj_b = np.asarray(proj_b, dtype=np.float32)

    wq2 = np.ascontiguousarray(np.transpose(w_q, (1, 0, 2)).reshape(DIM, H * DK))
    wk2 = np.ascontiguousarray(np.transpose(w_k, (1, 0, 2)).reshape(DIM, H * DK))
    wv2 = np.ascontiguousarray(np.transpose(w_v, (1, 0, 2)).reshape(DIM, H * DK))
    pwT = np.ascontiguousarray(proj_w.T).astype(ml_dtypes.bfloat16)
    pb2 = np.ascontiguousarray(proj_b.reshape(NDCH, P).T)

    in_maps = []
    for c in range(N_CORES):
        b, qo = c // 2, c % 2
        in_maps.append({
            "qT": np.ascontiguousarray(q[b, qo * TQ:(qo + 1) * TQ, :].T),
            "kT": np.ascontiguousarray(k[b].T),
            "vT": np.ascontiguousarray(v[b].T),
            "wq": wq2, "wk": wk2, "wv": wv2,
            "pwT": pwT, "pb": pb2,
        })

    res = run_bass_kernel_spmd(_NC, in_maps, list(range(N_CORES)), trace=TRACE)
    LAST_RESULT = res

    out = np.empty((B, L, DIM), dtype=np.float32)
    for c in range(N_CORES):
        b, qo = c // 2, c % 2
        out[b, qo * TQ:(qo + 1) * TQ, :] = res.results[c]["yT"].T
    return out


# revision 7
# speedup vs baseline: 1.4896x; 1.4896x over previous
"""Multi-head attention kernel for 8 Trainium2 NeuronCores.

Problem: B=4, L=2048, DIM=1024, H=16 heads, d_k=d_v=64.
  qh = q @ Wq_h ; kh = k @ Wk_h ; vh = v @ Wv_h          (per head)
  out_h = softmax(qh kh^T / sqrt(DIM)) vh
  y = concat_h(out_h) @ proj_w.T + proj_b

Sharding: data-parallel over (batch, query-half): core c handles batch
c//2, query rows [1024*(c%2), ...+1024). Each core holds the full K/V
for its batch -> no collectives; K/V projections are computed twice per
batch (12% extra flops) in exchange for zero comms.

Device dataflow (per core; contraction dim always on SBUF partitions;
all matmul inputs bf16, accumulation fp32 in PSUM). All matmuls are
K=128: the attention ones are made so by zero-padding the moving
operand, which costs SBUF but avoids the ~150ns/matmul penalty of
K=64 weight loads (no fast-weight-load path).

  phase V: VH'[st] [128s, 16h, 65] = vT-chunk.T @ wv   (+ ones column)
  phase Q: QHTz{p}[hp] [128, 1024q] = wq-chunk.T @ qT; parity-p head's
           64 rows live, sibling's 64 rows zeroed
  phase K: KHT[hp] [128, 2048s] = wk-chunk.T @ kT  (head pair stacked)
  phase B per head h (hp=h//2, p=h%2):
      S^T[key,q] = KHT[hp].T @ QHTz{p}[hp]  (zero rows kill the sibling
                                             head's contribution)
      expS = exp(S^T/32) -> bf16
      O'^T[65,q] = sum_st VH'[st][:,h,:].T @ expS[st]
      row 64 of O'^T = softmax denominator; reciprocal -> DRAM bounce ->
      partition-broadcast [64,512] -> oall[h][0:64] = O'^T[0:64] * recip
      (oall rows 64:127 stay zero)
  phase C: yT[dt] = sum_h pw_pad[h*64 : h*64+128, dt].T @ oall[h] + bias;
      oall's zero rows cancel the overlapping half of each 128-row weight
      chunk, so the contraction over the 1024 concat dims stays exact.
      pw is host-padded to 1088 rows.
"""

import numpy as np

P = 128
B, L, DIM, H, DK = 4, 2048, 1024, 16, 64
TQ = 1024      # q tokens per core
TS = 2048      # kv tokens per core
NDCH = DIM // P          # 8 contraction chunks
NHP = H // 2             # 8 head pairs
NST = TS // P            # 16 key tiles
N_CORES = 8

_NC = None
TRACE = False
LAST_RESULT = None


def _build():
    import concourse.bass as bass
    from concourse import bacc
    import concourse.mybir as mybir
    import concourse.tile as tile

    DT_B = mybir.dt.bfloat16
    DT_F = mybir.dt.float32
    AF = mybir.ActivationFunctionType

    nc = bacc.Bacc(None, target_bir_lowering=False)
    qT = nc.dram_tensor("qT", [DIM, TQ], DT_B, kind="ExternalInput")
    kT = nc.dram_tensor("kT", [DIM, TS], DT_B, kind="ExternalInput")
    vT = nc.dram_tensor("vT", [DIM, TS], DT_B, kind="ExternalInput")
    wq = nc.dram_tensor("wq", [DIM, H * DK], DT_B, kind="ExternalInput")
    wk = nc.dram_tensor("wk", [DIM, H * DK], DT_B, kind="ExternalInput")
    wv = nc.dram_tensor("wv", [DIM, H * DK], DT_B, kind="ExternalInput")
    pw = nc.dram_tensor("pwT", [H * DK + DK, DIM], DT_B, kind="ExternalInput")
    pb = nc.dram_tensor("pb", [P, NDCH], DT_F, kind="ExternalInput")
    yT = nc.dram_tensor("yT", [DIM, TQ], DT_F, kind="ExternalOutput")

    def bcast_ap(ap, count):
        return bass.AP(tensor=ap.tensor, offset=ap.offset,
                       ap=[[0, count]] + [list(x) for x in ap.ap[1:]])

    with tile.TileContext(nc) as tc, \
         tc.tile_pool(name="l1", bufs=1) as l1:
        # ---- whole-program tiles ----
        oall = [l1.tile([P, TQ], DT_B, name=f"oall{i}") for i in range(H)]
        pbt = l1.tile([P, NDCH], DT_F, name="pbt")
        nc.sync.dma_start(out=pbt[:, :], in_=pb[:, :])
        for h in range(H):
            nc.vector.memset(oall[h][DK:P, :], 0.0)

        with tc.tile_pool(name="l2", bufs=1) as l2:
            # ---- tiles that live through phase B ----
            qhtz = [[l2.tile([P, TQ], DT_B, name=f"qhtz{p}_{i}")
                     for i in range(NHP)] for p in range(2)]
            kht = [l2.tile([P, TS], DT_B, name=f"kht{i}") for i in range(NHP)]
            vhp = [l2.tile([P, H, DK + 1], DT_B, name=f"vhp{i}")
                   for i in range(NST)]
            for hp in range(NHP):
                nc.vector.memset(qhtz[0][hp][DK:P, :], 0.0)
                nc.vector.memset(qhtz[1][hp][0:DK, :], 0.0)

            # ---- phase V ----
            with tc.tile_pool(name="wv_pool", bufs=1) as wvp, \
                 tc.tile_pool(name="vslab", bufs=2) as vsp, \
                 tc.tile_pool(name="vpsum", bufs=2, space="PSUM") as vps:
                wvt = [wvp.tile([P, H * DK], DT_B, name=f"wvt{d}")
                       for d in range(NDCH)]
                for d in range(NDCH):
                    nc.sync.dma_start(out=wvt[d][:, :],
                                      in_=wv[d * P:(d + 1) * P, :])
                for st in range(NST):
                    vsl = [vsp.tile([P, P], DT_B, name=f"vsl{d}", tag=f"vsl{d}")
                           for d in range(NDCH)]
                    for d in range(NDCH):
                        nc.sync.dma_start(
                            out=vsl[d][:, :],
                            in_=vT[d * P:(d + 1) * P, st * P:(st + 1) * P])
                    ps = [vps.tile([P, 512], DT_F, name=f"vps_{st}_{n}",
                                   tag=f"vps{n}") for n in range(2)]
                    for d in range(NDCH):
                        for n in range(2):
                            nc.tensor.matmul(ps[n][:, :], vsl[d][:, :],
                                             wvt[d][:, n * 512:(n + 1) * 512],
                                             start=(d == 0), stop=(d == NDCH - 1))
                    for n in range(2):
                        nc.vector.tensor_copy(
                            vhp[st][:, n * 8:(n + 1) * 8, 0:DK],
                            ps[n][:, :].rearrange("p (h d) -> p h d", d=DK))
                    nc.vector.memset(vhp[st][:, :, DK:DK + 1], 1.0)

            # ---- phase Q ----
            with tc.tile_pool(name="qin_pool", bufs=1) as qip, \
                 tc.tile_pool(name="wq_pool", bufs=1) as wqp, \
                 tc.tile_pool(name="qpsum", bufs=2, space="PSUM") as qps:
                qin = [qip.tile([P, TQ], DT_B, name=f"qin{d}")
                       for d in range(NDCH)]
                wqt = [wqp.tile([P, H * DK], DT_B, name=f"wqt{d}")
                       for d in range(NDCH)]
                for d in range(NDCH):
                    nc.sync.dma_start(out=qin[d][:, :],
                                      in_=qT[d * P:(d + 1) * P, :])
                    nc.sync.dma_start(out=wqt[d][:, :],
                                      in_=wq[d * P:(d + 1) * P, :])
                for hp in range(NHP):
                    ps = [qps.tile([P, 512], DT_F, name=f"qps_{hp}_{n}",
                                   tag=f"qps{n}") for n in range(2)]
                    for d in range(NDCH):
                        for n in range(2):
                            nc.tensor.matmul(ps[n][:, :],
                                             wqt[d][:, hp * P:(hp + 1) * P],
                                             qin[d][:, n * 512:(n + 1) * 512],
                                             start=(d == 0), stop=(d == NDCH - 1))
                    for n in range(2):
                        nc.vector.tensor_copy(
                            qhtz[0][hp][0:DK, n * 512:(n + 1) * 512],
                            ps[n][0:DK, :])
                        nc.vector.tensor_copy(
                            qhtz[1][hp][DK:P, n * 512:(n + 1) * 512],
                            ps[n][DK:P, :])

            # ---- phase K ----
            with tc.tile_pool(name="kin_pool", bufs=1) as kip, \
                 tc.tile_pool(name="wk_pool", bufs=1) as wkp, \
                 tc.tile_pool(name="kpsum", bufs=2, space="PSUM") as kps:
                kin = [kip.tile([P, TS], DT_B, name=f"kin{d}")
                       for d in range(NDCH)]
                wkt = [wkp.tile([P, H * DK], DT_B, name=f"wkt{d}")
                       for d in range(NDCH)]
                for d in range(NDCH):
                    nc.scalar.dma_start(out=kin[d][:, :],
                                        in_=kT[d * P:(d + 1) * P, :])
                    nc.scalar.dma_start(out=wkt[d][:, :],
                                        in_=wk[d * P:(d + 1) * P, :])
                for hp in range(NHP):
                    ps = [kps.tile([P, 512], DT_F, name=f"kps_{hp}_{n}",
                                   tag=f"kps{n}") for n in range(4)]
                    for d in range(NDCH):
                        for n in range(4):
                            nc.tensor.matmul(ps[n][:, :],
                                             wkt[d][:, hp * P:(hp + 1) * P],
                                             kin[d][:, n * 512:(n + 1) * 512],
                                             start=(d == 0), stop=(d == NDCH - 1))
                    for n in range(4):
                        nc.vector.tensor_copy(
                            kht[hp][:, n * 512:(n + 1) * 512], ps[n][:, :])

            # ---- phase B: attention per head ----
            with tc.tile_pool(name="exp_pool", bufs=20) as expp, \
                 tc.tile_pool(name="spsum", bufs=2, space="PSUM") as sps, \
                 tc.tile_pool(name="opsum", bufs=4, space="PSUM") as ops, \
                 tc.tile_pool(name="sums_pool", bufs=4) as smp, \
                 tc.tile_pool(name="bc_pool", bufs=4) as bcp, \
                 tc.tile_pool(name="bounce", bufs=4, space="DRAM") as bncp:

                exp_tiles = [None] * H

                def emit_scores(h):
                    hp, p = h // 2, h % 2
                    tiles = []
                    for kt in range(NST):
                        sp = sps.tile([P, TQ], DT_F, name=f"sps_{h}_{kt}",
                                      tag="sps")
                        for n in range(2):
                            nc.tensor.matmul(
                                sp[:, n * 512:(n + 1) * 512],
                                kht[hp][:, kt * P:(kt + 1) * P],
                                qhtz[p][hp][:, n * 512:(n + 1) * 512],
                                start=True, stop=True)
                        ex = expp.tile([P, TQ], DT_B, name=f"exp_{h}_{kt}",
                                       tag="exp")
                        nc.scalar.activation(ex[:, :], sp[:, :], AF.Exp,
                                             scale=1.0 / 32.0)
                        tiles.append(ex)
                    exp_tiles[h] = tiles

                def emit_pv(h):
                    tiles = exp_tiles[h]
                    for qh in range(2):
                        op = ops.tile([P, 512], DT_F, name=f"ops_{h}_{qh}",
                                      tag="ops")
                        for kt in range(NST):
                            nc.tensor.matmul(
                                op[0:DK + 1, :], vhp[kt][:, h, :],
                                tiles[kt][:, qh * 512:(qh + 1) * 512],
                                start=(kt == 0), stop=(kt == NST - 1))
                        sm = smp.tile([DK + 1, 512], DT_F, name=f"sm_{h}_{qh}",
                                      tag="sm")
                        nc.vector.reciprocal(sm[DK:DK + 1, :], op[DK:DK + 1, :])
                        bn = bncp.tile([1, 512], DT_F, name=f"bn_{h}_{qh}",
                                       tag="bn")
                        nc.sync.dma_start(out=bn[:, :], in_=sm[DK:DK + 1, :])
                        bc = bcp.tile([DK, 512], DT_F, name=f"bc_{h}_{qh}",
                                      tag="bc")
                        nc.sync.dma_start(out=bc[:, :],
                                          in_=bcast_ap(bn[0:1, :], DK))
                        nc.vector.tensor_mul(oall[h][0:DK, qh * 512:(qh + 1) * 512],
                                             op[0:DK, :], bc[:, :])
                    exp_tiles[h] = None

                emit_scores(0)
                for h in range(H):
                    if h + 1 < H:
                        emit_scores(h + 1)
                    emit_pv(h)

        # ---- phase C (l2 closed; SBUF free) ----
        with tc.tile_pool(name="pw_pool", bufs=1) as pwp, \
             tc.tile_pool(name="ypsum", bufs=2, space="PSUM") as yps, \
             tc.tile_pool(name="yst_pool", bufs=4) as ystp:
            pwsb = [pwp.tile([P, DIM], DT_B, name=f"pwsb{h}") for h in range(H)]
            for h in range(H):
                nc.scalar.dma_start(out=pwsb[h][:, :],
                                    in_=pw[h * DK:h * DK + P, :])
            for dt_ in range(NDCH):
                ps = [yps.tile([P, 512], DT_F, name=f"yps_{dt_}_{n}",
                               tag=f"yps{n}") for n in range(2)]
                for h in range(H):
                    for n in range(2):
                        nc.tensor.matmul(ps[n][:, :],
                                         pwsb[h][:, dt_ * P:(dt_ + 1) * P],
                                         oall[h][:, n * 512:(n + 1) * 512],
                                         start=(h == 0), stop=(h == H - 1))
                for n in range(2):
                    yst = ystp.tile([P, 512], DT_F, name=f"yst_{dt_}_{n}",
                                    tag="yst")
                    nc.vector.tensor_scalar_add(yst[:, :], ps[n][:, :],
                                                pbt[:, dt_:dt_ + 1])
                    nc.sync.dma_start(
                        out=yT[dt_ * P:(dt_ + 1) * P, n * 512:(n + 1) * 512],
                        in_=yst[:, :])

    nc.compile()
    return nc


def kernel(q, k, v, w_q, w_k, w_v, proj_w, proj_b):
    global _NC, LAST_RESULT
    import ml_dtypes
    from concourse.bass_utils import run_bass_kernel_spmd

    if _NC is None:
        _NC = _build()

    bf16 = ml_dtypes.bfloat16
    q = np.asarray(q, dtype=np.float32)
    k = np.asarray(k, dtype=np.float32)
    v = np.asarray(v, dtype=np.float32)
    w_q = np.asarray(w_q, dtype=np.float32)
    w_k = np.asarray(w_k, dtype=np.float32)
    w_v = np.asarray(w_v, dtype=np.float32)
    proj_w = np.asarray(proj_w, dtype=np.float32)
    proj_b = np.asarray(proj_b, dtype=np.float32)

    wq2 = np.ascontiguousarray(
        np.transpose(w_q, (1, 0, 2)).reshape(DIM, H * DK)).astype(bf16)
    wk2 = np.ascontiguousarray(
        np.transpose(w_k, (1, 0, 2)).reshape(DIM, H * DK)).astype(bf16)
    wv2 = np.ascontiguousarray(
        np.transpose(w_v, (1, 0, 2)).reshape(DIM, H * DK)).astype(bf16)
    pwT = np.zeros((H * DK + DK, DIM), dtype=bf16)
    pwT[0:H * DK] = np.ascontiguousarray(proj_w.T).astype(bf16)
    pb2 = np.ascontiguousarray(proj_b.reshape(NDCH, P).T)

    in_maps = []
    for c in range(N_CORES):
        b, qo = c // 2, c % 2
        in_maps.append({
            "qT": np.ascontiguousarray(
                q[b, qo * TQ:(qo + 1) * TQ, :].T).astype(bf16),
            "kT": np.ascontiguousarray(k[b].T).astype(bf16),
            "vT": np.ascontiguousarray(v[b].T).astype(bf16),
            "wq": wq2, "wk": wk2, "wv": wv2,
            "pwT": pwT, "pb": pb2,
        })

    res = run_bass_kernel_spmd(_NC, in_maps, list(range(N_CORES)), trace=TRACE)
    LAST_RESULT = res

    out = np.empty((B, L, DIM), dtype=np.float32)
    for c in range(N_CORES):
        b, qo = c // 2, c % 2
        out[b, qo * TQ:(qo + 1) * TQ, :] = res.results[c]["yT"].T
    return out


# revision 8
# speedup vs baseline: 1.5275x; 1.0254x over previous
"""Multi-head attention kernel for 8 Trainium2 NeuronCores.

Problem: B=4, L=2048, DIM=1024, H=16 heads, d_k=d_v=64.
  qh = q @ Wq_h ; kh = k @ Wk_h ; vh = v @ Wv_h          (per head)
  out_h = softmax(qh kh^T / sqrt(DIM)) vh
  y = concat_h(out_h) @ proj_w.T + proj_b

Sharding: data-parallel over (batch, query-half): core c handles batch
c//2, query rows [1024*(c%2), ...+1024). Each core holds the full K/V
for its batch -> no collectives; K/V projections are computed twice per
batch (12% extra flops) in exchange for zero comms.

Device dataflow (per core; contraction dim always on SBUF partitions;
all matmul inputs bf16, accumulation fp32 in PSUM). All matmuls are
K=128: the attention ones are made so by zero-padding the moving
operand, which costs SBUF but avoids the ~150ns/matmul penalty of
K=64 weight loads (no fast-weight-load path).

  phase V: VH'[st] [128s, 16h, 65] = vT-chunk.T @ wv   (+ ones column)
  phase Q: QHTz{p}[hp] [128, 1024q] = wq-chunk.T @ qT; parity-p head's
           64 rows live, sibling's 64 rows zeroed
  phase K: KHT[hp] [128, 2048s] = wk-chunk.T @ kT  (head pair stacked)
  phase B per head h (hp=h//2, p=h%2):
      S^T[key,q] = KHT[hp].T @ QHTz{p}[hp]  (zero rows kill the sibling
                                             head's contribution)
      expS = exp(S^T/32) -> bf16
      O'^T[65,q] = sum_st VH'[st][:,h,:].T @ expS[st]
      row 64 of O'^T = softmax denominator; reciprocal -> DRAM bounce ->
      partition-broadcast [64,512] -> oall[h][0:64] = O'^T[0:64] * recip
      (oall rows 64:127 stay zero)
  phase C: yT[dt] = sum_h pw_pad[h*64 : h*64+128, dt].T @ oall[h] + bias;
      oall's zero rows cancel the overlapping half of each 128-row weight
      chunk, so the contraction over the 1024 concat dims stays exact.
      pw is host-padded to 1088 rows.
"""

import numpy as np

P = 128
B, L, DIM, H, DK = 4, 2048, 1024, 16, 64
TQ = 1024      # q tokens per core
TS = 2048      # kv tokens per core
NDCH = DIM // P          # 8 contraction chunks
NHP = H // 2             # 8 head pairs
NST = TS // P            # 16 key tiles
N_CORES = 8

_NC = None
TRACE = False
LAST_RESULT = None


def _build():
    import concourse.bass as bass
    from concourse import bacc
    import concourse.mybir as mybir
    import concourse.tile as tile

    DT_B = mybir.dt.bfloat16
    DT_F = mybir.dt.float32
    AF = mybir.ActivationFunctionType

    nc = bacc.Bacc(None, target_bir_lowering=False)
    qT = nc.dram_tensor("qT", [DIM, TQ], DT_B, kind="ExternalInput")
    kT = nc.dram_tensor("kT", [DIM, TS], DT_B, kind="ExternalInput")
    vT = nc.dram_tensor("vT", [DIM, TS], DT_B, kind="ExternalInput")
    wq = nc.dram_tensor("wq", [DIM, H * DK], DT_B, kind="ExternalInput")
    wk = nc.dram_tensor("wk", [DIM, H * DK], DT_B, kind="ExternalInput")
    wv = nc.dram_tensor("wv", [DIM, H * DK], DT_B, kind="ExternalInput")
    pw = nc.dram_tensor("pwT", [H * DK + DK, DIM], DT_B, kind="ExternalInput")
    pb = nc.dram_tensor("pb", [P, NDCH], DT_F, kind="ExternalInput")
    yT = nc.dram_tensor("yT", [DIM, TQ], DT_F, kind="ExternalOutput")

    def bcast_ap(ap, count):
        return bass.AP(tensor=ap.tensor, offset=ap.offset,
                       ap=[[0, count]] + [list(x) for x in ap.ap[1:]])

    with tile.TileContext(nc) as tc, \
         tc.tile_pool(name="l1", bufs=1) as l1:
        # ---- whole-program tiles ----
        oall = [l1.tile([P, TQ], DT_B, name=f"oall{i}") for i in range(H)]
        pbt = l1.tile([P, NDCH], DT_F, name="pbt")
        nc.sync.dma_start(out=pbt[:, :], in_=pb[:, :])
        for h in range(H):
            nc.vector.memset(oall[h][DK:P, :], 0.0)

        with tc.tile_pool(name="l2", bufs=1) as l2:
            # ---- tiles that live through phase B ----
            qhtz = [[l2.tile([P, TQ], DT_B, name=f"qhtz{p}_{i}")
                     for i in range(NHP)] for p in range(2)]
            kht = [l2.tile([P, TS], DT_B, name=f"kht{i}") for i in range(NHP)]
            vhp = [l2.tile([P, H, DK + 1], DT_B, name=f"vhp{i}")
                   for i in range(NST)]
            for hp in range(NHP):
                nc.vector.memset(qhtz[0][hp][DK:P, :], 0.0)
                nc.vector.memset(qhtz[1][hp][0:DK, :], 0.0)

            # ---- phase V ----
            with tc.tile_pool(name="wv_pool", bufs=1) as wvp, \
                 tc.tile_pool(name="vin_pool", bufs=1) as vip, \
                 tc.tile_pool(name="vpsum", bufs=2, space="PSUM") as vps:
                wvt = [wvp.tile([P, H * DK], DT_B, name=f"wvt{d}")
                       for d in range(NDCH)]
                vin = [vip.tile([P, TS], DT_B, name=f"vin{d}")
                       for d in range(NDCH)]
                for d in range(NDCH):
                    nc.sync.dma_start(out=wvt[d][:, :],
                                      in_=wv[d * P:(d + 1) * P, :])
                    nc.sync.dma_start(out=vin[d][:, :],
                                      in_=vT[d * P:(d + 1) * P, :])
                for st in range(NST):
                    ps = [vps.tile([P, 512], DT_F, name=f"vps_{st}_{n}",
                                   tag=f"vps{n}") for n in range(2)]
                    for d in range(NDCH):
                        for n in range(2):
                            nc.tensor.matmul(ps[n][:, :],
                                             vin[d][:, st * P:(st + 1) * P],
                                             wvt[d][:, n * 512:(n + 1) * 512],
                                             start=(d == 0), stop=(d == NDCH - 1))
                    for n in range(2):
                        nc.vector.tensor_copy(
                            vhp[st][:, n * 8:(n + 1) * 8, 0:DK],
                            ps[n][:, :].rearrange("p (h d) -> p h d", d=DK))
                    nc.vector.memset(vhp[st][:, :, DK:DK + 1], 1.0)

            # ---- phase Q ----
            with tc.tile_pool(name="qin_pool", bufs=1) as qip, \
                 tc.tile_pool(name="wq_pool", bufs=1) as wqp, \
                 tc.tile_pool(name="qpsum", bufs=2, space="PSUM") as qps:
                qin = [qip.tile([P, TQ], DT_B, name=f"qin{d}")
                       for d in range(NDCH)]
                wqt = [wqp.tile([P, H * DK], DT_B, name=f"wqt{d}")
                       for d in range(NDCH)]
                for d in range(NDCH):
                    nc.sync.dma_start(out=qin[d][:, :],
                                      in_=qT[d * P:(d + 1) * P, :])
                    nc.sync.dma_start(out=wqt[d][:, :],
                                      in_=wq[d * P:(d + 1) * P, :])
                for hp in range(NHP):
                    ps = [qps.tile([P, 512], DT_F, name=f"qps_{hp}_{n}",
                                   tag=f"qps{n}") for n in range(2)]
                    for d in range(NDCH):
                        for n in range(2):
                            nc.tensor.matmul(ps[n][:, :],
                                             wqt[d][:, hp * P:(hp + 1) * P],
                                             qin[d][:, n * 512:(n + 1) * 512],
                                             start=(d == 0), stop=(d == NDCH - 1))
                    for n in range(2):
                        nc.vector.tensor_copy(
                            qhtz[0][hp][0:DK, n * 512:(n + 1) * 512],
                            ps[n][0:DK, :])
                        nc.vector.tensor_copy(
                            qhtz[1][hp][DK:P, n * 512:(n + 1) * 512],
                            ps[n][DK:P, :])

            # ---- phase K ----
            with tc.tile_pool(name="kin_pool", bufs=1) as kip, \
                 tc.tile_pool(name="wk_pool", bufs=1) as wkp, \
                 tc.tile_pool(name="kpsum", bufs=2, space="PSUM") as kps:
                kin = [kip.tile([P, TS], DT_B, name=f"kin{d}")
                       for d in range(NDCH)]
                wkt = [wkp.tile([P, H * DK], DT_B, name=f"wkt{d}")
                       for d in range(NDCH)]
                for d in range(NDCH):
                    nc.scalar.dma_start(out=kin[d][:, :],
                                        in_=kT[d * P:(d + 1) * P, :])
                    nc.scalar.dma_start(out=wkt[d][:, :],
                                        in_=wk[d * P:(d + 1) * P, :])
                for hp in range(NHP):
                    ps = [kps.tile([P, 512], DT_F, name=f"kps_{hp}_{n}",
                                   tag=f"kps{n}") for n in range(4)]
                    for d in range(NDCH):
                        for n in range(4):
                            nc.tensor.matmul(ps[n][:, :],
                                             wkt[d][:, hp * P:(hp + 1) * P],
                                             kin[d][:, n * 512:(n + 1) * 512],
                                             start=(d == 0), stop=(d == NDCH - 1))
                    for n in range(4):
                        nc.vector.tensor_copy(
                            kht[hp][:, n * 512:(n + 1) * 512], ps[n][:, :])

            # ---- phase B: attention per head ----
            with tc.tile_pool(name="exp_pool", bufs=20) as expp, \
                 tc.tile_pool(name="spsum", bufs=2, space="PSUM") as sps, \
                 tc.tile_pool(name="opsum", bufs=4, space="PSUM") as ops, \
                 tc.tile_pool(name="sums_pool", bufs=4) as smp, \
                 tc.tile_pool(name="bc_pool", bufs=4) as bcp, \
                 tc.tile_pool(name="bounce", bufs=4, space="DRAM") as bncp:

                exp_tiles = [None] * H

                def emit_scores(h):
                    hp, p = h // 2, h % 2
                    tiles = []
                    for kt in range(NST):
                        sp = sps.tile([P, TQ], DT_F, name=f"sps_{h}_{kt}",
                                      tag="sps")
                        for n in range(2):
                            nc.tensor.matmul(
                                sp[:, n * 512:(n + 1) * 512],
                                kht[hp][:, kt * P:(kt + 1) * P],
                                qhtz[p][hp][:, n * 512:(n + 1) * 512],
                                start=True, stop=True)
                        ex = expp.tile([P, TQ], DT_B, name=f"exp_{h}_{kt}",
                                       tag="exp")
                        nc.scalar.activation(ex[:, :], sp[:, :], AF.Exp,
                                             scale=1.0 / 32.0)
                        tiles.append(ex)
                    exp_tiles[h] = tiles

                def emit_pv(h):
                    tiles = exp_tiles[h]
                    for qh in range(2):
                        op = ops.tile([P, 512], DT_F, name=f"ops_{h}_{qh}",
                                      tag="ops")
                        for kt in range(NST):
                            nc.tensor.matmul(
                                op[0:DK + 1, :], vhp[kt][:, h, :],
                                tiles[kt][:, qh * 512:(qh + 1) * 512],
                                start=(kt == 0), stop=(kt == NST - 1))
                        sm = smp.tile([DK + 1, 512], DT_F, name=f"sm_{h}_{qh}",
                                      tag="sm")
                        nc.vector.reciprocal(sm[DK:DK + 1, :], op[DK:DK + 1, :])
                        bn = bncp.tile([1, 512], DT_F, name=f"bn_{h}_{qh}",
                                       tag="bn")
                        nc.sync.dma_start(out=bn[:, :], in_=sm[DK:DK + 1, :])
                        bc = bcp.tile([DK, 512], DT_F, name=f"bc_{h}_{qh}",
                                      tag="bc")
                        nc.sync.dma_start(out=bc[:, :],
                                          in_=bcast_ap(bn[0:1, :], DK))
                        nc.vector.tensor_mul(oall[h][0:DK, qh * 512:(qh + 1) * 512],
                                             op[0:DK, :], bc[:, :])
                    exp_tiles[h] = None

                emit_scores(0)
                for h in range(H):
                    if h + 1 < H:
                        emit_scores(h + 1)
                    emit_pv(h)

        # ---- phase C (l2 closed; SBUF free) ----
        with tc.tile_pool(name="pw_pool", bufs=1) as pwp, \
             tc.tile_pool(name="ypsum", bufs=2, space="PSUM") as yps, \
             tc.tile_pool(name="yst_pool", bufs=4) as ystp:
            pwsb = [pwp.tile([P, DIM], DT_B, name=f"pwsb{h}") for h in range(H)]
            for h in range(H):
                nc.scalar.dma_start(out=pwsb[h][:, :],
                                    in_=pw[h * DK:h * DK + P, :])
            for dt_ in range(NDCH):
                ps = [yps.tile([P, 512], DT_F, name=f"yps_{dt_}_{n}",
                               tag=f"yps{n}") for n in range(2)]
                for h in range(H):
                    for n in range(2):
                        nc.tensor.matmul(ps[n][:, :],
                                         pwsb[h][:, dt_ * P:(dt_ + 1) * P],
                                         oall[h][:, n * 512:(n + 1) * 512],
                                         start=(h == 0), stop=(h == H - 1))
                for n in range(2):
                    yst = ystp.tile([P, 512], DT_F, name=f"yst_{dt_}_{n}",
                                    tag="yst")
                    nc.vector.tensor_scalar_add(yst[:, :], ps[n][:, :],
                                                pbt[:, dt_:dt_ + 1])
                    nc.sync.dma_start(
                        out=yT[dt_ * P:(dt_ + 1) * P, n * 512:(n + 1) * 512],
                        in_=yst[:, :])

    nc.compile()
    return nc


def kernel(q, k, v, w_q, w_k, w_v, proj_w, proj_b):
    global _NC, LAST_RESULT
    import ml_dtypes
    from concourse.bass_utils import run_bass_kernel_spmd

    if _NC is None:
        _NC = _build()

    bf16 = ml_dtypes.bfloat16
    q = np.asarray(q, dtype=np.float32)
    k = np.asarray(k, dtype=np.float32)
    v = np.asarray(v, dtype=np.float32)
    w_q = np.asarray(w_q, dtype=np.float32)
    w_k = np.asarray(w_k, dtype=np.float32)
    w_v = np.asarray(w_v, dtype=np.float32)
    proj_w = np.asarray(proj_w, dtype=np.float32)
    proj_b = np.asarray(proj_b, dtype=np.float32)

    wq2 = np.ascontiguousarray(
        np.transpose(w_q, (1, 0, 2)).reshape(DIM, H * DK)).astype(bf16)
    wk2 = np.ascontiguousarray(
        np.transpose(w_k, (1, 0, 2)).reshape(DIM, H * DK)).astype(bf16)
    wv2 = np.ascontiguousarray(
        np.transpose(w_v, (1, 0, 2)).reshape(DIM, H * DK)).astype(bf16)
    pwT = np.zeros((H * DK + DK, DIM), dtype=bf16)
    pwT[0:H * DK] = np.ascontiguousarray(proj_w.T).astype(bf16)
    pb2 = np.ascontiguousarray(proj_b.reshape(NDCH, P).T)

    in_maps = []
    for c in range(N_CORES):
        b, qo = c // 2, c % 2
        in_maps.append({
            "qT": np.ascontiguousarray(
                q[b, qo * TQ:(qo + 1) * TQ, :].T).astype(bf16),
            "kT": np.ascontiguousarray(k[b].T).astype(bf16),
            "vT": np.ascontiguousarray(v[b].T).astype(bf16),
            "wq": wq2, "wk": wk2, "wv": wv2,
            "pwT": pwT, "pb": pb2,
        })

    res = run_bass_kernel_spmd(_NC, in_maps, list(range(N_CORES)), trace=TRACE)
    LAST_RESULT = res

    out = np.empty((B, L, DIM), dtype=np.float32)
    for c in range(N_CORES):
        b, qo = c // 2, c % 2
        out[b, qo * TQ:(qo + 1) * TQ, :] = res.results[c]["yT"].T
    return out


# revision 9
# speedup vs baseline: 1.5551x; 1.0181x over previous
"""Multi-head attention kernel for 8 Trainium2 NeuronCores.

Problem: B=4, L=2048, DIM=1024, H=16 heads, d_k=d_v=64.
  qh = q @ Wq_h ; kh = k @ Wk_h ; vh = v @ Wv_h          (per head)
  out_h = softmax(qh kh^T / sqrt(DIM)) vh
  y = concat_h(out_h) @ proj_w.T + proj_b

Sharding: data-parallel over (batch, query-half): core c handles batch
c//2, query rows [1024*(c%2), ...+1024). Each core holds the full K/V
for its batch -> no collectives; K/V projections are computed twice per
batch (12% extra flops) in exchange for zero comms.

Device dataflow (per core; contraction dim always on SBUF partitions;
all matmul inputs bf16, accumulation fp32 in PSUM). All matmuls are
K=128: the attention ones are made so by zero-padding the moving
operand, which costs SBUF but avoids the ~150ns/matmul penalty of
K=64 weight loads (no fast-weight-load path).

  phase V: VH'[st] [128s, 16h, 65] = vT-chunk.T @ wv   (+ ones column)
  phase Q: QHTz{p}[hp] [128, 1024q] = wq-chunk.T @ qT; parity-p head's
           64 rows live, sibling's 64 rows zeroed
  phase K: KHT[hp] [128, 2048s] = wk-chunk.T @ kT  (head pair stacked)
  phase B per head h (hp=h//2, p=h%2):
      S^T[key,q] = KHT[hp].T @ QHTz{p}[hp]  (zero rows kill the sibling
                                             head's contribution)
      expS = exp(S^T/32) -> bf16
      O'^T[65,q] = sum_st VH'[st][:,h,:].T @ expS[st]
      row 64 of O'^T = softmax denominator; reciprocal -> DRAM bounce ->
      partition-broadcast [64,512] -> oall[h][0:64] = O'^T[0:64] * recip
      (oall rows 64:127 stay zero)
  phase C: yT[dt] = sum_h pw_pad[h*64 : h*64+128, dt].T @ oall[h] + bias;
      oall's zero rows cancel the overlapping half of each 128-row weight
      chunk, so the contraction over the 1024 concat dims stays exact.
      pw is host-padded to 1088 rows.
"""

import numpy as np

P = 128
B, L, DIM, H, DK = 4, 2048, 1024, 16, 64
TQ = 1024      # q tokens per core
TS = 2048      # kv tokens per core
NDCH = DIM // P          # 8 contraction chunks
NHP = H // 2             # 8 head pairs
NST = TS // P            # 16 key tiles
N_CORES = 8

_NC = None
TRACE = False
LAST_RESULT = None


def _build():
    import concourse.bass as bass
    from concourse import bacc
    import concourse.mybir as mybir
    import concourse.tile as tile

    DT_B = mybir.dt.bfloat16
    DT_F = mybir.dt.float32
    AF = mybir.ActivationFunctionType

    nc = bacc.Bacc(None, target_bir_lowering=False)
    qT = nc.dram_tensor("qT", [DIM, TQ], DT_B, kind="ExternalInput")
    kT = nc.dram_tensor("kTh", [DIM, TS // 2], DT_B, kind="ExternalInput")
    vT = nc.dram_tensor("vTh", [DIM, TS // 2], DT_B, kind="ExternalInput")
    wq = nc.dram_tensor("wq", [DIM, H * DK], DT_B, kind="ExternalInput")
    wk = nc.dram_tensor("wk", [DIM, H * DK], DT_B, kind="ExternalInput")
    wv = nc.dram_tensor("wv", [DIM, H * DK], DT_B, kind="ExternalInput")
    pw = nc.dram_tensor("pwT", [H * DK + DK, DIM], DT_B, kind="ExternalInput")
    pb = nc.dram_tensor("pb", [P, NDCH], DT_F, kind="ExternalInput")
    yT = nc.dram_tensor("yT", [DIM, TQ], DT_F, kind="ExternalOutput")

    def bcast_ap(ap, count):
        return bass.AP(tensor=ap.tensor, offset=ap.offset,
                       ap=[[0, count]] + [list(x) for x in ap.ap[1:]])

    with tile.TileContext(nc) as tc, \
         tc.tile_pool(name="l1", bufs=1) as l1:
        # ---- whole-program tiles ----
        oall = [l1.tile([P, TQ], DT_B, name=f"oall{i}") for i in range(H)]
        pbt = l1.tile([P, NDCH], DT_F, name="pbt")
        nc.sync.dma_start(out=pbt[:, :], in_=pb[:, :])
        for h in range(H):
            nc.vector.memset(oall[h][DK:P, :], 0.0)

        with tc.tile_pool(name="l2", bufs=1) as l2:
            # ---- tiles that live through phase B ----
            qhtz = [[l2.tile([P, TQ], DT_B, name=f"qhtz{p}_{i}")
                     for i in range(NHP)] for p in range(2)]
            kht = [l2.tile([P, TS], DT_B, name=f"kht{i}") for i in range(NHP)]
            vhp = [l2.tile([P, H, DK + 1], DT_B, name=f"vhp{i}")
                   for i in range(NST)]
            for hp in range(NHP):
                nc.vector.memset(qhtz[0][hp][DK:P, :], 0.0)
                nc.vector.memset(qhtz[1][hp][0:DK, :], 0.0)

            # ---- phase V: this core computes its key-half's 8 s-tiles,
            # the pair partner computes the other 8; AllGather merges ----
            with tc.tile_pool(name="wv_pool", bufs=1) as wvp, \
                 tc.tile_pool(name="vin_pool", bufs=1) as vip, \
                 tc.tile_pool(name="vst_pool", bufs=3) as vstp, \
                 tc.tile_pool(name="vdram", bufs=1, space="DRAM") as vdp, \
                 tc.tile_pool(name="vpsum", bufs=2, space="PSUM") as vps:
                wvt = [wvp.tile([P, H * DK], DT_B, name=f"wvt{d}")
                       for d in range(NDCH)]
                vin = [vip.tile([P, TS // 2], DT_B, name=f"vin{d}")
                       for d in range(NDCH)]
                for d in range(NDCH):
                    nc.sync.dma_start(out=wvt[d][:, :],
                                      in_=wv[d * P:(d + 1) * P, :])
                    nc.sync.dma_start(out=vin[d][:, :],
                                      in_=vT[d * P:(d + 1) * P, :])
                vout = vdp.tile([NST // 2, P, H * (DK + 1)], DT_B, name="vout")
                vgath = vdp.tile([2, NST // 2, P, H * (DK + 1)], DT_B,
                                 name="vgath")
                for st in range(NST // 2):
                    ps = [vps.tile([P, 512], DT_F, name=f"vps_{st}_{n}",
                                   tag=f"vps{n}") for n in range(2)]
                    for d in range(NDCH):
                        for n in range(2):
                            nc.tensor.matmul(ps[n][:, :],
                                             vin[d][:, st * P:(st + 1) * P],
                                             wvt[d][:, n * 512:(n + 1) * 512],
                                             start=(d == 0), stop=(d == NDCH - 1))
                    vst = vstp.tile([P, H, DK + 1], DT_B, name=f"vst{st}",
                                    tag="vst")
                    for n in range(2):
                        nc.vector.tensor_copy(
                            vst[:, n * 8:(n + 1) * 8, 0:DK],
                            ps[n][:, :].rearrange("p (h d) -> p h d", d=DK))
                    nc.vector.memset(vst[:, :, DK:DK + 1], 1.0)
                    nc.sync.dma_start(
                        out=vout[st, :, :],
                        in_=vst[:, :, :].rearrange("p h d -> p (h d)"))
                nc.gpsimd.collective_compute(
                    "AllGather", mybir.AluOpType.bypass,
                    replica_groups=[[0, 1], [2, 3], [4, 5], [6, 7]],
                    ins=[vout[:, :, :]], outs=[vgath[:, :, :, :]])
                for half in range(2):
                    for st in range(NST // 2):
                        nc.sync.dma_start(
                            out=vhp[half * (NST // 2) + st][:, :, :],
                            in_=vgath[half, st, :, :].rearrange(
                                "p (h d) -> p h d", d=DK + 1))

            # ---- phase Q ----
            with tc.tile_pool(name="qin_pool", bufs=1) as qip, \
                 tc.tile_pool(name="wq_pool", bufs=1) as wqp, \
                 tc.tile_pool(name="qpsum", bufs=2, space="PSUM") as qps:
                qin = [qip.tile([P, TQ], DT_B, name=f"qin{d}")
                       for d in range(NDCH)]
                wqt = [wqp.tile([P, H * DK], DT_B, name=f"wqt{d}")
                       for d in range(NDCH)]
                for d in range(NDCH):
                    nc.sync.dma_start(out=qin[d][:, :],
                                      in_=qT[d * P:(d + 1) * P, :])
                    nc.sync.dma_start(out=wqt[d][:, :],
                                      in_=wq[d * P:(d + 1) * P, :])
                for hp in range(NHP):
                    ps = [qps.tile([P, 512], DT_F, name=f"qps_{hp}_{n}",
                                   tag=f"qps{n}") for n in range(2)]
                    for d in range(NDCH):
                        for n in range(2):
                            nc.tensor.matmul(ps[n][:, :],
                                             wqt[d][:, hp * P:(hp + 1) * P],
                                             qin[d][:, n * 512:(n + 1) * 512],
                                             start=(d == 0), stop=(d == NDCH - 1))
                    for n in range(2):
                        nc.vector.tensor_copy(
                            qhtz[0][hp][0:DK, n * 512:(n + 1) * 512],
                            ps[n][0:DK, :])
                        nc.vector.tensor_copy(
                            qhtz[1][hp][DK:P, n * 512:(n + 1) * 512],
                            ps[n][DK:P, :])

            # ---- phase K: this core projects its key-half; per-head-pair
            # AllGather merges the two halves into kht ----
            with tc.tile_pool(name="kin_pool", bufs=1) as kip, \
                 tc.tile_pool(name="wk_pool", bufs=1) as wkp, \
                 tc.tile_pool(name="kst_pool", bufs=3) as kstp, \
                 tc.tile_pool(name="kdram", bufs=1, space="DRAM") as kdp, \
                 tc.tile_pool(name="kpsum", bufs=2, space="PSUM") as kps:
                kin = [kip.tile([P, TS // 2], DT_B, name=f"kin{d}")
                       for d in range(NDCH)]
                wkt = [wkp.tile([P, H * DK], DT_B, name=f"wkt{d}")
                       for d in range(NDCH)]
                for d in range(NDCH):
                    nc.scalar.dma_start(out=kin[d][:, :],
                                        in_=kT[d * P:(d + 1) * P, :])
                    nc.scalar.dma_start(out=wkt[d][:, :],
                                        in_=wk[d * P:(d + 1) * P, :])
                kout = [kdp.tile([P, TS // 2], DT_B, name=f"kout{hp}")
                        for hp in range(NHP)]
                kgath = [kdp.tile([2, P, TS // 2], DT_B, name=f"kgath{hp}")
                         for hp in range(NHP)]
                for hp in range(NHP):
                    ps = [kps.tile([P, 512], DT_F, name=f"kps_{hp}_{n}",
                                   tag=f"kps{n}") for n in range(2)]
                    for d in range(NDCH):
                        for n in range(2):
                            nc.tensor.matmul(ps[n][:, :],
                                             wkt[d][:, hp * P:(hp + 1) * P],
                                             kin[d][:, n * 512:(n + 1) * 512],
                                             start=(d == 0), stop=(d == NDCH - 1))
                    kst = kstp.tile([P, TS // 2], DT_B, name=f"kst{hp}",
                                    tag="kst")
                    for n in range(2):
                        nc.vector.tensor_copy(
                            kst[:, n * 512:(n + 1) * 512], ps[n][:, :])
                    nc.sync.dma_start(out=kout[hp][:, :], in_=kst[:, :])
                    nc.gpsimd.collective_compute(
                        "AllGather", mybir.AluOpType.bypass,
                        replica_groups=[[0, 1], [2, 3], [4, 5], [6, 7]],
                        ins=[kout[hp][:, :]], outs=[kgath[hp][:, :, :]])
                    for half in range(2):
                        nc.sync.dma_start(
                            out=kht[hp][:, half * (TS // 2):(half + 1) * (TS // 2)],
                            in_=kgath[hp][half, :, :])

            # ---- phase B: attention per head ----
            with tc.tile_pool(name="exp_pool", bufs=20) as expp, \
                 tc.tile_pool(name="spsum", bufs=2, space="PSUM") as sps, \
                 tc.tile_pool(name="opsum", bufs=4, space="PSUM") as ops, \
                 tc.tile_pool(name="sums_pool", bufs=4) as smp, \
                 tc.tile_pool(name="bc_pool", bufs=4) as bcp, \
                 tc.tile_pool(name="bounce", bufs=4, space="DRAM") as bncp:

                exp_tiles = [None] * H

                def emit_scores(h):
                    hp, p = h // 2, h % 2
                    tiles = []
                    for kt in range(NST):
                        sp = sps.tile([P, TQ], DT_F, name=f"sps_{h}_{kt}",
                                      tag="sps")
                        for n in range(2):
                            nc.tensor.matmul(
                                sp[:, n * 512:(n + 1) * 512],
                                kht[hp][:, kt * P:(kt + 1) * P],
                                qhtz[p][hp][:, n * 512:(n + 1) * 512],
                                start=True, stop=True)
                        ex = expp.tile([P, TQ], DT_B, name=f"exp_{h}_{kt}",
                                       tag="exp")
                        nc.scalar.activation(ex[:, :], sp[:, :], AF.Exp,
                                             scale=1.0 / 32.0)
                        tiles.append(ex)
                    exp_tiles[h] = tiles

                def emit_pv(h):
                    tiles = exp_tiles[h]
                    for qh in range(2):
                        op = ops.tile([P, 512], DT_F, name=f"ops_{h}_{qh}",
                                      tag="ops")
                        for kt in range(NST):
                            nc.tensor.matmul(
                                op[0:DK + 1, :], vhp[kt][:, h, :],
                                tiles[kt][:, qh * 512:(qh + 1) * 512],
                                start=(kt == 0), stop=(kt == NST - 1))
                        sm = smp.tile([DK + 1, 512], DT_F, name=f"sm_{h}_{qh}",
                                      tag="sm")
                        nc.vector.reciprocal(sm[DK:DK + 1, :], op[DK:DK + 1, :])
                        bn = bncp.tile([1, 512], DT_F, name=f"bn_{h}_{qh}",
                                       tag="bn")
                        nc.sync.dma_start(out=bn[:, :], in_=sm[DK:DK + 1, :])
                        bc = bcp.tile([DK, 512], DT_F, name=f"bc_{h}_{qh}",
                                      tag="bc")
                        nc.sync.dma_start(out=bc[:, :],
                                          in_=bcast_ap(bn[0:1, :], DK))
                        nc.vector.tensor_mul(oall[h][0:DK, qh * 512:(qh + 1) * 512],
                                             op[0:DK, :], bc[:, :])
                    exp_tiles[h] = None

                emit_scores(0)
                for h in range(H):
                    if h + 1 < H:
                        emit_scores(h + 1)
                    emit_pv(h)

        # ---- phase C (l2 closed; SBUF free) ----
        with tc.tile_pool(name="pw_pool", bufs=1) as pwp, \
             tc.tile_pool(name="ypsum", bufs=2, space="PSUM") as yps, \
             tc.tile_pool(name="yst_pool", bufs=4) as ystp:
            pwsb = [pwp.tile([P, DIM], DT_B, name=f"pwsb{h}") for h in range(H)]
            for h in range(H):
                nc.scalar.dma_start(out=pwsb[h][:, :],
                                    in_=pw[h * DK:h * DK + P, :])
            for dt_ in range(NDCH):
                ps = [yps.tile([P, 512], DT_F, name=f"yps_{dt_}_{n}",
                               tag=f"yps{n}") for n in range(2)]
                for h in range(H):
                    for n in range(2):
                        nc.tensor.matmul(ps[n][:, :],
                                         pwsb[h][:, dt_ * P:(dt_ + 1) * P],
                                         oall[h][:, n * 512:(n + 1) * 512],
                                         start=(h == 0), stop=(h == H - 1))
                for n in range(2):
                    yst = ystp.tile([P, 512], DT_F, name=f"yst_{dt_}_{n}",
                                    tag="yst")
                    nc.vector.tensor_scalar_add(yst[:, :], ps[n][:, :],
                                                pbt[:, dt_:dt_ + 1])
                    nc.sync.dma_start(
                        out=yT[dt_ * P:(dt_ + 1) * P, n * 512:(n + 1) * 512],
                        in_=yst[:, :])

    nc.compile()
    return nc


def kernel(q, k, v, w_q, w_k, w_v, proj_w, proj_b):
    global _NC, LAST_RESULT
    import ml_dtypes
    from concourse.bass_utils import run_bass_kernel_spmd

    if _NC is None:
        _NC = _build()

    bf16 = ml_dtypes.bfloat16
    q = np.asarray(q, dtype=np.float32)
    k = np.asarray(k, dtype=np.float32)
    v = np.asarray(v, dtype=np.float32)
    w_q = np.asarray(w_q, dtype=np.float32)
    w_k = np.asarray(w_k, dtype=np.float32)
    w_v = np.asarray(w_v, dtype=np.float32)
    proj_w = np.asarray(proj_w, dtype=np.float32)
    proj_b = np.asarray(proj_b, dtype=np.float32)

    wq2 = np.ascontiguousarray(
        np.transpose(w_q, (1, 0, 2)).reshape(DIM, H * DK)).astype(bf16)
    wk2 = np.ascontiguousarray(
        np.transpose(w_k, (1, 0, 2)).reshape(DIM, H * DK)).astype(bf16)
    wv2 = np.ascontiguousarray(
        np.transpose(w_v, (1, 0, 2)).reshape(DIM, H * DK)).astype(bf16)
    pwT = np.zeros((H * DK + DK, DIM), dtype=bf16)
    pwT[0:H * DK] = np.ascontiguousarray(proj_w.T).astype(bf16)
    pb2 = np.ascontiguousarray(proj_b.reshape(NDCH, P).T)

    in_maps = []
    for c in range(N_CORES):
        b, qo = c // 2, c % 2
        in_maps.append({
            "qT": np.ascontiguousarray(
                q[b, qo * TQ:(qo + 1) * TQ, :].T).astype(bf16),
            "kTh": np.ascontiguousarray(
                k[b, qo * TQ:(qo + 1) * TQ, :].T).astype(bf16),
            "vTh": np.ascontiguousarray(
                v[b, qo * TQ:(qo + 1) * TQ, :].T).astype(bf16),
            "wq": wq2, "wk": wk2, "wv": wv2,
            "pwT": pwT, "pb": pb2,
        })

    res = run_bass_kernel_spmd(_NC, in_maps, list(range(N_CORES)), trace=TRACE)
    LAST_RESULT = res

    out = np.empty((B, L, DIM), dtype=np.float32)
    for c in range(N_CORES):
        b, qo = c // 2, c % 2
        out[b, qo * TQ:(qo + 1) * TQ, :] = res.results[c]["yT"].T
    return out
